# revision 34
# baseline (speedup 1.0000x reference)
"""MoE actor kernel for 8 TRN2 NeuronCores (expert-parallel, host routing).

Problem: B=65536 tokens, obs dim D=376, each routed by `o` to one of E=8
experts; per-expert MLP 376 -> 256 -> 256 -> {mean[17], log_std[17]} with
relu/relu/(identity|tanh-affine) heads.

Strategy: routing/gather happens on the host (numpy) — core e receives
exactly the tokens assigned to expert e (padded to 16 full 512-token tiles
plus one short remainder tile) and only that expert's weights. Every core
runs the same dense 3-layer MLP graph with features on the partition axis:

    h1T[H, n] = relu(W1.T @ xT + b1)     K=384(pad of 376) -> M=256
    h2T[H, n] = relu(W2.T @ h1T + b2)    K=256 -> M=256
    zT[64, n] = Wc.T @ h2T               K=256 -> M=64 (mean @0:17, z @32:49)
    rows 32:49 -> tanh(z + bs) on ScalarE

Matmuls run in bf16 (full PE rate, FWL weight loads; ~4e-3 rel err, fp32
PSUM accumulate). L3+epilogue are deferred one tile so the PE never stalls;
ReLUs alternate between ScalarE and VectorE. The host scatters per-core
outputs back to original token order, adding the mean bias and the
log-std affine (3.5*t - 1.5) during the scatter.
"""

import numpy as np

B, D, H, A, E = 65536, 376, 256, 17, 8
DPAD = 384          # D padded to 3 partition tiles of 128
TOK = 512           # token tile (matmul free dim; one PSUM bank)
AOUT = 2 * A        # 34: mean ++ log_std

# test.py hooks: set TRACE=True before calling kernel() to profile; the
# BassKernelResults of the last run lands in LAST_RESULT.
TRACE = False
TRACE_CORES = None
LAST_RESULT = None

_cache = {}


def _install_axon_ntff_hook():
    """antenv.axon_hooks is absent in this image; recreate it so
    run_bass_kernel_spmd(trace=True) can capture NTFF profiles."""
    import sys, types
    if 'antenv.axon_hooks' in sys.modules:
        return
    try:
        from trn_agent_boot.trn_boot import _ntff_profile_via_ctypes
        hook = _ntff_profile_via_ctypes('/opt/axon/libaxon_pjrt.so')
    except Exception:
        hook = None
    m = types.ModuleType('antenv.axon_hooks')
    m.get_axon_ntff_profile_hook = lambda: hook
    m.set_axon_ntff_profile_hook = lambda h: None
    sys.modules['antenv.axon_hooks'] = m


def _build(n_full, rem):
    import concourse.bass as bass
    import concourse.tile as tile
    from concourse import bacc, mybir

    f32 = mybir.dt.float32
    f32r = mybir.dt.float32r
    bf16 = mybir.dt.bfloat16
    AF = mybir.ActivationFunctionType
    ds = bass.ds
    sizes = [TOK] * n_full + ([rem] if rem else [])
    npad = n_full * TOK + rem
    offs = [i * TOK for i in range(len(sizes))]

    nc = bacc.Bacc("TRN2", target_bir_lowering=False, debug=False, num_devices=E)
    x_ext = nc.dram_tensor("x", [128, 3 * npad], bf16, kind="ExternalInput")
    w1_ext = nc.dram_tensor("w1", [128, 3 * H], bf16, kind="ExternalInput")
    rest_ext = nc.dram_tensor("rest", [128, 768], bf16, kind="ExternalInput")
    bias_ext = nc.dram_tensor("bias", [128, 5], f32, kind="ExternalInput")
    out_ext = nc.dram_tensor("out", [AOUT, npad], f32, kind="ExternalOutput")

    with tile.TileContext(nc) as tc:
        with tc.tile_pool(name="wp", bufs=1) as wp, \
             tc.tile_pool(name="xp", bufs=6) as xp, \
             tc.tile_pool(name="hp", bufs=3) as hp, \
             tc.tile_pool(name="op", bufs=4) as op, \
             tc.tile_pool(name="ps1", bufs=1, space="PSUM") as ps1, \
             tc.tile_pool(name="ps2", bufs=1, space="PSUM") as ps2, \
             tc.tile_pool(name="ps3", bufs=2, space="PSUM") as ps3:
            # PE pre-warm: dummy matmuls with no input deps keep the HAM
            # activity window busy while the first x tiles stream in, so the
            # real matmul phase starts at 2.4 GHz instead of 1.2.
            warm_w = wp.tile([128, 128], f32, name="warm_w")
            nc.gpsimd.memset(warm_w[:], 0.0)
            warm_x = wp.tile([128, TOK], f32, name="warm_x")
            nc.gpsimd.memset(warm_x[:], 0.0)
            pw = ps3.tile([128, TOK], f32, tag="pwarm", name="pwarm")
            for _ in range(3):
                nc.tensor.matmul(pw[:], warm_w[:], warm_x[:],
                                 start=True, stop=True)

            w1 = wp.tile([128, 3 * H], bf16)
            rest = wp.tile([128, 768], bf16)
            bias = wp.tile([128, 5], f32)
            w2 = rest[:, ds(0, 2 * H)]
            wc = rest[:, ds(2 * H, 2 * 128)]
            b1 = bias[:, ds(0, 2)]
            b2 = bias[:, ds(2, 2)]
            bc = bias[:, ds(4, 1)]

            def head_tail(t, h2):
                # L3 + epilogue for tile t (deferred one iteration so the
                # PE rolls straight into the next tile's L1/L2). Mean rows
                # leave PSUM raw (host adds bm); log_std rows get
                # tanh(z + bs) here and the affine on the host.
                n = sizes[t]
                off = offs[t]
                p3 = ps3.tile([128, TOK], f32, tag="p3")
                for k in range(2):
                    nc.tensor.matmul(
                        p3[:, 0:n], wc[:, ds(k * 128, 128)], h2[k][:, 0:n],
                        start=(k == 0), stop=(k == 1))
                ot = op.tile([64, TOK], f32, tag="ot")
                nc.vector.tensor_copy(ot[0:A, 0:n], p3[0:A, 0:n])
                nc.scalar.activation(ot[32:32 + A, 0:n], p3[32:32 + A, 0:n],
                                     AF.Tanh, bias=bc[32:32 + A, :])
                nc.gpsimd.dma_start(out_ext.ap()[0:A, off:off + n],
                                    ot[0:A, 0:n])
                nc.gpsimd.dma_start(out_ext.ap()[A:AOUT, off:off + n],
                                    ot[32:32 + A, 0:n])

            prev = None
            for t, n in enumerate(sizes):
                xoff = 3 * offs[t]
                xsb = xp.tile([128, 3 * TOK], bf16, tag="x")
                if t == 0:
                    # First tile: split the x transfer so the first matmul
                    # only waits on the k=0 piece, and interleave the weight
                    # transfers so they aren't stuck behind the x backlog.
                    nc.sync.dma_start(xsb[:, ds(0, n)],
                                      x_ext.ap()[:, xoff:xoff + n])
                    nc.sync.dma_start(w1[:], w1_ext.ap()[:])
                    nc.sync.dma_start(rest[:], rest_ext.ap()[:])
                    nc.sync.dma_start(xsb[:, ds(n, n)],
                                      x_ext.ap()[:, xoff + n:xoff + 2 * n])
                    nc.sync.dma_start(xsb[:, ds(2 * n, n)],
                                      x_ext.ap()[:, xoff + 2 * n:xoff + 3 * n])
                    nc.sync.dma_start(bias[:], bias_ext.ap()[:])
                else:
                    nc.sync.dma_start(xsb[:, 0:3 * n],
                                      x_ext.ap()[:, xoff:xoff + 3 * n])
                xk = [xsb[:, ds(k * n, n)] for k in range(3)]

                p1 = [ps1.tile([128, TOK], f32, tag=f"p1_{m}", name=f"p1_{m}")
                      for m in range(2)]
                if t == 0:
                    km_order = [(k, m) for k in range(3) for m in range(2)]
                else:
                    km_order = [(k, m) for m in range(2) for k in range(3)]
                for k, m in km_order:
                    nc.tensor.matmul(
                        p1[m][:, 0:n], w1[:, ds(k * H + m * 128, 128)],
                        xk[k], start=(k == 0), stop=(k == 2))
                h1 = []
                for m in range(2):
                    h = hp.tile([128, TOK], bf16, tag=f"h1_{m}")
                    if m == 0:
                        nc.scalar.activation(h[:, 0:n], p1[m][:, 0:n], AF.Relu,
                                             bias=b1[:, ds(m, 1)])
                    else:
                        nc.vector.tensor_scalar(
                            out=h[:, 0:n], in0=p1[m][:, 0:n],
                            scalar1=b1[:, ds(m, 1)], scalar2=0.0,
                            op0=mybir.AluOpType.add, op1=mybir.AluOpType.max)
                    h1.append(h)

                if prev is not None:
                    head_tail(prev[0], prev[1])

                # k-major order: the k=0 matmuls only need h1[0], giving the
                # engine producing h1[1] time to finish.
                p2 = [ps2.tile([128, TOK], f32, tag=f"p2_{m}", name=f"p2_{m}")
                      for m in range(2)]
                for k in range(2):
                    for m in range(2):
                        nc.tensor.matmul(
                            p2[m][:, 0:n], w2[:, ds(k * H + m * 128, 128)],
                            h1[k][:, 0:n],
                            start=(k == 0), stop=(k == 1))
                h2 = []
                for m in range(2):
                    h = hp.tile([128, TOK], bf16, tag=f"h2_{m}")
                    if m == 0:
                        nc.scalar.activation(h[:, 0:n], p2[m][:, 0:n], AF.Relu,
                                             bias=b2[:, ds(m, 1)])
                    else:
                        nc.vector.tensor_scalar(
                            out=h[:, 0:n], in0=p2[m][:, 0:n],
                            scalar1=b2[:, ds(m, 1)], scalar2=0.0,
                            op0=mybir.AluOpType.add, op1=mybir.AluOpType.max)
                    h2.append(h)

                prev = (t, h2)
            head_tail(prev[0], prev[1])

    nc.compile()
    return nc


def _get_compiled(n_full, rem):
    key = (n_full, rem)
    nc = _cache.get(key)
    if nc is None:
        nc = _build(n_full, rem)
        _cache[key] = nc
    return nc


def kernel(x, o, W1, b1, W2, b2, Wm, bm, Ws, bs):
    global LAST_RESULT
    from concourse import bass_utils

    x = np.asarray(x, dtype=np.float32)
    o_i = np.asarray(o).astype(np.int64)
    W1 = np.asarray(W1, dtype=np.float32)
    b1 = np.asarray(b1, dtype=np.float32)
    W2 = np.asarray(W2, dtype=np.float32)
    b2 = np.asarray(b2, dtype=np.float32)
    Wm = np.asarray(Wm, dtype=np.float32)
    bm = np.asarray(bm, dtype=np.float32)
    Ws = np.asarray(Ws, dtype=np.float32)
    bs = np.asarray(bs, dtype=np.float32)

    nb, d = x.shape
    counts = np.bincount(o_i, minlength=E)
    cmax = int(counts.max())
    n_full = max(1, cmax // TOK)
    rem = -(-max(0, cmax - n_full * TOK) // 128) * 128
    npad = n_full * TOK + rem
    order = np.argsort(o_i, kind="stable")
    idx_per_e = np.split(order, np.cumsum(counts)[:-1])
    sizes = [TOK] * n_full + ([rem] if rem else [])
    offs = [i * TOK for i in range(len(sizes))]

    in_maps = []
    for e in range(E):
        idx = idx_per_e[e]
        import ml_dtypes
        xg = np.zeros((npad, DPAD), ml_dtypes.bfloat16)
        xg[:len(idx), :d] = x[idx].astype(ml_dtypes.bfloat16)
        x_pack = np.concatenate(
            [xg[off:off + n].reshape(n, 3, 128).transpose(2, 1, 0).reshape(
                128, 3 * n) for off, n in zip(offs, sizes)], axis=1)
        x_pack = np.ascontiguousarray(x_pack)

        w1p = np.zeros((DPAD, H), np.float32)
        w1p[:d] = W1[e]
        w1_pack = np.ascontiguousarray(
            w1p.reshape(3, 128, H).transpose(1, 0, 2)).reshape(128, 3 * H)
        w2_pack = np.ascontiguousarray(
            W2[e].reshape(2, 128, H).transpose(1, 0, 2)).reshape(128, 2 * H)
        wc_full = np.zeros((H, 128), np.float32)
        wc_full[:, 0:A] = Wm[e]
        wc_full[:, 32:32 + A] = Ws[e]
        wc_pack = np.ascontiguousarray(
            wc_full.reshape(2, 128, 128).transpose(1, 0, 2)).reshape(
                128, 2 * 128)
        b1_pack = np.ascontiguousarray(b1[e].reshape(2, 128).T)
        b2_pack = np.ascontiguousarray(b2[e].reshape(2, 128).T)
        bc_pack = np.zeros((128, 1), np.float32)
        bc_pack[32:32 + A, 0] = bs[e]
        import ml_dtypes
        rest_pack = np.concatenate(
            [w2_pack, wc_pack], axis=1).astype(ml_dtypes.bfloat16)
        bias_pack = np.concatenate([b1_pack, b2_pack, bc_pack], axis=1)

        in_maps.append({"x": x_pack, "w1": w1_pack.astype(ml_dtypes.bfloat16),
                        "rest": rest_pack, "bias": bias_pack})

    nc = _get_compiled(n_full, rem)

    kwargs = {}
    if TRACE:
        _install_axon_ntff_hook()
        bass_utils.upload_artifacts = lambda tmpdir: f"local:{tmpdir}"
        kwargs["trace"] = True
        if TRACE_CORES is not None:
            kwargs["trace_cores"] = TRACE_CORES
    res = None
    for attempt in range(3):
        try:
            res = bass_utils.run_bass_kernel_spmd(
                nc, in_maps, core_ids=list(range(E)), **kwargs)
            break
        except Exception:
            if attempt == 2:
                raise
            import time
            time.sleep(15)
    LAST_RESULT = res

    mean = np.empty((nb, A), np.float32)
    log_std = np.empty((nb, A), np.float32)
    for e in range(E):
        out = res.results[e]["out"]                       # [34, npad]
        ofull = out.T
        idx = idx_per_e[e]
        mean[idx] = ofull[:len(idx), :A] + bm[e]
        log_std[idx] = 3.5 * ofull[:len(idx), A:AOUT] - 1.5
    return mean, log_std


# revision 35
# speedup vs baseline: 1.0049x; 1.0049x over previous
"""MoE actor kernel for 8 TRN2 NeuronCores (expert-parallel, host routing).

Problem: B=65536 tokens, obs dim D=376, each routed by `o` to one of E=8
experts; per-expert MLP 376 -> 256 -> 256 -> {mean[17], log_std[17]} with
relu/relu/(identity|tanh-affine) heads.

Strategy: routing/gather happens on the host (numpy) — core e receives
exactly the tokens assigned to expert e (padded to 16 full 512-token tiles
plus one short remainder tile) and only that expert's weights. Every core
runs the same dense 3-layer MLP graph with features on the partition axis:

    h1T[H, n] = relu(W1.T @ xT + b1)     K=384(pad of 376) -> M=256
    h2T[H, n] = relu(W2.T @ h1T + b2)    K=256 -> M=256
    zT[64, n] = Wc.T @ h2T               K=256 -> M=64 (mean @0:17, z @32:49)
    rows 32:49 -> tanh(z + bs) on ScalarE

Matmuls run in bf16 (full PE rate, FWL weight loads; ~4e-3 rel err, fp32
PSUM accumulate). L3+epilogue are deferred one tile so the PE never stalls;
ReLUs alternate between ScalarE and VectorE. The host scatters per-core
outputs back to original token order, adding the mean bias and the
log-std affine (3.5*t - 1.5) during the scatter.
"""

import numpy as np

B, D, H, A, E = 65536, 376, 256, 17, 8
DPAD = 384          # D padded to 3 partition tiles of 128
TOK = 512           # token tile (matmul free dim; one PSUM bank)
AOUT = 2 * A        # 34: mean ++ log_std

# test.py hooks: set TRACE=True before calling kernel() to profile; the
# BassKernelResults of the last run lands in LAST_RESULT.
TRACE = False
TRACE_CORES = None
LAST_RESULT = None

_cache = {}


def _install_axon_ntff_hook():
    """antenv.axon_hooks is absent in this image; recreate it so
    run_bass_kernel_spmd(trace=True) can capture NTFF profiles."""
    import sys, types
    if 'antenv.axon_hooks' in sys.modules:
        return
    try:
        from trn_agent_boot.trn_boot import _ntff_profile_via_ctypes
        hook = _ntff_profile_via_ctypes('/opt/axon/libaxon_pjrt.so')
    except Exception:
        hook = None
    m = types.ModuleType('antenv.axon_hooks')
    m.get_axon_ntff_profile_hook = lambda: hook
    m.set_axon_ntff_profile_hook = lambda h: None
    sys.modules['antenv.axon_hooks'] = m


def _build(n_full, rem):
    import concourse.bass as bass
    import concourse.tile as tile
    from concourse import bacc, mybir

    f32 = mybir.dt.float32
    f32r = mybir.dt.float32r
    bf16 = mybir.dt.bfloat16
    AF = mybir.ActivationFunctionType
    ds = bass.ds
    sizes = [TOK] * n_full + ([rem] if rem else [])
    npad = n_full * TOK + rem
    offs = [i * TOK for i in range(len(sizes))]

    nc = bacc.Bacc("TRN2", target_bir_lowering=False, debug=False, num_devices=E)
    x_ext = nc.dram_tensor("x", [128, 3 * npad], bf16, kind="ExternalInput")
    w1_ext = nc.dram_tensor("w1", [128, 3 * H], bf16, kind="ExternalInput")
    rest_ext = nc.dram_tensor("rest", [128, 768], bf16, kind="ExternalInput")
    bias_ext = nc.dram_tensor("bias", [128, 5], f32, kind="ExternalInput")
    out_ext = nc.dram_tensor("out", [AOUT, npad], f32, kind="ExternalOutput")

    with tile.TileContext(nc) as tc:
        with tc.tile_pool(name="wp", bufs=1) as wp, \
             tc.tile_pool(name="xp", bufs=6) as xp, \
             tc.tile_pool(name="hp", bufs=3) as hp, \
             tc.tile_pool(name="op", bufs=4) as op, \
             tc.tile_pool(name="ps1", bufs=1, space="PSUM") as ps1, \
             tc.tile_pool(name="ps2", bufs=1, space="PSUM") as ps2, \
             tc.tile_pool(name="ps3", bufs=2, space="PSUM") as ps3:
            # PE pre-warm: dummy matmuls with no input deps keep the HAM
            # activity window busy while the first x tiles stream in, so the
            # real matmul phase starts at 2.4 GHz instead of 1.2.
            warm_w = wp.tile([128, 128], f32, name="warm_w")
            nc.gpsimd.memset(warm_w[:], 0.0)
            warm_x = wp.tile([128, TOK], f32, name="warm_x")
            nc.gpsimd.memset(warm_x[:], 0.0)
            pw = ps3.tile([128, TOK], f32, tag="pwarm", name="pwarm")
            for _ in range(3):
                nc.tensor.matmul(pw[:], warm_w[:], warm_x[:],
                                 start=True, stop=True)

            w1 = wp.tile([128, 3 * H], bf16)
            rest = wp.tile([128, 768], bf16)
            bias = wp.tile([128, 5], f32)
            w2 = rest[:, ds(0, 2 * H)]
            wc = rest[:, ds(2 * H, 2 * 128)]
            b1 = bias[:, ds(0, 2)]
            b2 = bias[:, ds(2, 2)]
            bc = bias[:, ds(4, 1)]

            def head_tail(t, h2):
                # L3 + epilogue for tile t (deferred one iteration so the
                # PE rolls straight into the next tile's L1/L2). Mean rows
                # leave PSUM raw (host adds bm); log_std rows get
                # tanh(z + bs) here and the affine on the host.
                n = sizes[t]
                off = offs[t]
                p3 = ps3.tile([128, TOK], f32, tag="p3")
                for k in range(2):
                    nc.tensor.matmul(
                        p3[:, 0:n], wc[:, ds(k * 128, 128)], h2[k][:, 0:n],
                        start=(k == 0), stop=(k == 1))
                ot = op.tile([64, TOK], f32, tag="ot")
                nc.vector.tensor_copy(ot[0:A, 0:n], p3[0:A, 0:n])
                nc.scalar.activation(ot[32:32 + A, 0:n], p3[32:32 + A, 0:n],
                                     AF.Tanh, bias=bc[32:32 + A, :])
                nc.gpsimd.dma_start(out_ext.ap()[0:A, off:off + n],
                                    ot[0:A, 0:n])
                nc.gpsimd.dma_start(out_ext.ap()[A:AOUT, off:off + n],
                                    ot[32:32 + A, 0:n])

            prev = None
            for t, n in enumerate(sizes):
                xoff = 3 * offs[t]
                xsb = xp.tile([128, 3 * TOK], bf16, tag="x")
                if t == 0:
                    # First tile: split the x transfer so the first matmul
                    # only waits on the k=0 piece, and interleave the weight
                    # transfers so they aren't stuck behind the x backlog.
                    nc.sync.dma_start(xsb[:, ds(0, n)],
                                      x_ext.ap()[:, xoff:xoff + n])
                    nc.scalar.dma_start(w1[:], w1_ext.ap()[:])
                    nc.sync.dma_start(xsb[:, ds(n, n)],
                                      x_ext.ap()[:, xoff + n:xoff + 2 * n])
                    nc.scalar.dma_start(rest[:], rest_ext.ap()[:])
                    nc.sync.dma_start(xsb[:, ds(2 * n, n)],
                                      x_ext.ap()[:, xoff + 2 * n:xoff + 3 * n])
                    nc.scalar.dma_start(bias[:], bias_ext.ap()[:])
                else:
                    nc.sync.dma_start(xsb[:, 0:3 * n],
                                      x_ext.ap()[:, xoff:xoff + 3 * n])
                xk = [xsb[:, ds(k * n, n)] for k in range(3)]

                p1 = [ps1.tile([128, TOK], f32, tag=f"p1_{m}", name=f"p1_{m}")
                      for m in range(2)]
                if t == 0:
                    km_order = [(k, m) for k in range(3) for m in range(2)]
                else:
                    km_order = [(k, m) for m in range(2) for k in range(3)]
                for k, m in km_order:
                    nc.tensor.matmul(
                        p1[m][:, 0:n], w1[:, ds(k * H + m * 128, 128)],
                        xk[k], start=(k == 0), stop=(k == 2))
                h1 = []
                for m in range(2):
                    h = hp.tile([128, TOK], bf16, tag=f"h1_{m}")
                    if m == 0:
                        nc.scalar.activation(h[:, 0:n], p1[m][:, 0:n], AF.Relu,
                                             bias=b1[:, ds(m, 1)])
                    else:
                        nc.vector.tensor_scalar(
                            out=h[:, 0:n], in0=p1[m][:, 0:n],
                            scalar1=b1[:, ds(m, 1)], scalar2=0.0,
                            op0=mybir.AluOpType.add, op1=mybir.AluOpType.max)
                    h1.append(h)

                if prev is not None:
                    head_tail(prev[0], prev[1])

                # k-major order: the k=0 matmuls only need h1[0], giving the
                # engine producing h1[1] time to finish.
                p2 = [ps2.tile([128, TOK], f32, tag=f"p2_{m}", name=f"p2_{m}")
                      for m in range(2)]
                for k in range(2):
                    for m in range(2):
                        nc.tensor.matmul(
                            p2[m][:, 0:n], w2[:, ds(k * H + m * 128, 128)],
                            h1[k][:, 0:n],
                            start=(k == 0), stop=(k == 1))
                h2 = []
                for m in range(2):
                    h = hp.tile([128, TOK], bf16, tag=f"h2_{m}")
                    if m == 0:
                        nc.scalar.activation(h[:, 0:n], p2[m][:, 0:n], AF.Relu,
                                             bias=b2[:, ds(m, 1)])
                    else:
                        nc.vector.tensor_scalar(
                            out=h[:, 0:n], in0=p2[m][:, 0:n],
                            scalar1=b2[:, ds(m, 1)], scalar2=0.0,
                            op0=mybir.AluOpType.add, op1=mybir.AluOpType.max)
                    h2.append(h)

                prev = (t, h2)
            head_tail(prev[0], prev[1])

    nc.compile()
    return nc


def _get_compiled(n_full, rem):
    key = (n_full, rem)
    nc = _cache.get(key)
    if nc is None:
        nc = _build(n_full, rem)
        _cache[key] = nc
    return nc


def kernel(x, o, W1, b1, W2, b2, Wm, bm, Ws, bs):
    global LAST_RESULT
    from concourse import bass_utils

    x = np.asarray(x, dtype=np.float32)
    o_i = np.asarray(o).astype(np.int64)
    W1 = np.asarray(W1, dtype=np.float32)
    b1 = np.asarray(b1, dtype=np.float32)
    W2 = np.asarray(W2, dtype=np.float32)
    b2 = np.asarray(b2, dtype=np.float32)
    Wm = np.asarray(Wm, dtype=np.float32)
    bm = np.asarray(bm, dtype=np.float32)
    Ws = np.asarray(Ws, dtype=np.float32)
    bs = np.asarray(bs, dtype=np.float32)

    nb, d = x.shape
    counts = np.bincount(o_i, minlength=E)
    cmax = int(counts.max())
    n_full = max(1, cmax // TOK)
    rem = -(-max(0, cmax - n_full * TOK) // 128) * 128
    npad = n_full * TOK + rem
    order = np.argsort(o_i, kind="stable")
    idx_per_e = np.split(order, np.cumsum(counts)[:-1])
    sizes = [TOK] * n_full + ([rem] if rem else [])
    offs = [i * TOK for i in range(len(sizes))]

    in_maps = []
    for e in range(E):
        idx = idx_per_e[e]
        import ml_dtypes
        xg = np.zeros((npad, DPAD), ml_dtypes.bfloat16)
        xg[:len(idx), :d] = x[idx].astype(ml_dtypes.bfloat16)
        x_pack = np.concatenate(
            [xg[off:off + n].reshape(n, 3, 128).transpose(2, 1, 0).reshape(
                128, 3 * n) for off, n in zip(offs, sizes)], axis=1)
        x_pack = np.ascontiguousarray(x_pack)

        w1p = np.zeros((DPAD, H), np.float32)
        w1p[:d] = W1[e]
        w1_pack = np.ascontiguousarray(
            w1p.reshape(3, 128, H).transpose(1, 0, 2)).reshape(128, 3 * H)
        w2_pack = np.ascontiguousarray(
            W2[e].reshape(2, 128, H).transpose(1, 0, 2)).reshape(128, 2 * H)
        wc_full = np.zeros((H, 128), np.float32)
        wc_full[:, 0:A] = Wm[e]
        wc_full[:, 32:32 + A] = Ws[e]
        wc_pack = np.ascontiguousarray(
            wc_full.reshape(2, 128, 128).transpose(1, 0, 2)).reshape(
                128, 2 * 128)
        b1_pack = np.ascontiguousarray(b1[e].reshape(2, 128).T)
        b2_pack = np.ascontiguousarray(b2[e].reshape(2, 128).T)
        bc_pack = np.zeros((128, 1), np.float32)
        bc_pack[32:32 + A, 0] = bs[e]
        import ml_dtypes
        rest_pack = np.concatenate(
            [w2_pack, wc_pack], axis=1).astype(ml_dtypes.bfloat16)
        bias_pack = np.concatenate([b1_pack, b2_pack, bc_pack], axis=1)

        in_maps.append({"x": x_pack, "w1": w1_pack.astype(ml_dtypes.bfloat16),
                        "rest": rest_pack, "bias": bias_pack})

    nc = _get_compiled(n_full, rem)

    kwargs = {}
    if TRACE:
        _install_axon_ntff_hook()
        bass_utils.upload_artifacts = lambda tmpdir: f"local:{tmpdir}"
        kwargs["trace"] = True
        if TRACE_CORES is not None:
            kwargs["trace_cores"] = TRACE_CORES
    res = None
    for attempt in range(3):
        try:
            res = bass_utils.run_bass_kernel_spmd(
                nc, in_maps, core_ids=list(range(E)), **kwargs)
            break
        except Exception:
            if attempt == 2:
                raise
            import time
            time.sleep(15)
    LAST_RESULT = res

    mean = np.empty((nb, A), np.float32)
    log_std = np.empty((nb, A), np.float32)
    for e in range(E):
        out = res.results[e]["out"]                       # [34, npad]
        ofull = out.T
        idx = idx_per_e[e]
        mean[idx] = ofull[:len(idx), :A] + bm[e]
        log_std[idx] = 3.5 * ofull[:len(idx), A:AOUT] - 1.5
    return mean, log_std
